# revision 10
# baseline (speedup 1.0000x reference)
"""Trainium2 Bass kernel for the BMN-style pooling network.

Contract: kernel(**inputs) takes the FULL unsharded inputs (as produced by
setup_inputs) and returns the full outputs (matching reference()).

Sharding: pure data-parallel, 8 cores = 4 batches x 2 halves of the t2 (end
index) axis. All weights replicated; the big sample_mask is re-laid-out on
the host into per-core streaming-friendly slices. One SPMD program; all
per-core variation is carried in the input data.

Math per core (batch b, t2 range of 64 cols + 2-col halo each side = 68
"slots"):
  h   = relu(conv1d(relu(conv1d(xr, w1)), w2))           (128 ch x 128 t)
  x4  = sigmoid(0.01*(w3 @ h + b3))
  f_s = w3d[:, :, s] @ h          for s in 0..31          (512 o x 128 t)
  cm[o, t1, t2] = sum_{s,t} f[o, s, t] * mask[t, s, t1, t2]   (PE matmuls,
        mask streamed as moving operand, N = 4 slots x 128 t1 = 512)
  p = relu(wa @ relu(cm + b3d) + ba)       -> dense (c x slot x t1) plane
  q = relu(conv3x3(p, wb) + bb); r = relu(conv3x3(q, wc) + bc)
  prop = sigmoid(wd @ r + bd)
"""

import numpy as np

T = 128
S = 32
NSLOT = 68          # 64 output t2 cols + 2 halo each side
NG = NSLOT // 4     # BM groups of 4 slots (N = 512)
COLW = 130          # per-slot column width in the dense planes (t1 0..127 + 2 pads)
PLANE = NSLOT * COLW
MARGIN = 132        # tail margin so shifted conv reads stay in-bounds

_CACHE = {}


def _build_program():
    import concourse.bacc as bacc
    import concourse.tile as tile
    from concourse import mybir

    F32R = mybir.dt.float32r
    FP32 = mybir.dt.float32
    AF = mybir.ActivationFunctionType

    nc = bacc.Bacc("TRN2", target_bir_lowering=False)

    # ---------------- DRAM I/O (per-core content) ----------------
    x_d = nc.dram_tensor("x", [200, T], F32R, kind="ExternalInput")
    w1t_d = nc.dram_tensor("w1t", [3, 200, 512], F32R, kind="ExternalInput")
    w2t_d = nc.dram_tensor("w2t", [3, 512, 128], F32R, kind="ExternalInput")
    w3t_d = nc.dram_tensor("w3t", [128, 3], F32R, kind="ExternalInput")
    w3dt_d = nc.dram_tensor("w3dt", [S, 128, 512], F32R, kind="ExternalInput")
    wat_d = nc.dram_tensor("wat", [512, 128], F32R, kind="ExternalInput")
    wbt_d = nc.dram_tensor("wbt", [9, 128, 64], F32R, kind="ExternalInput")
    wct_d = nc.dram_tensor("wct", [9, 64, 64], F32R, kind="ExternalInput")
    wdt_d = nc.dram_tensor("wdt", [64, 4], F32R, kind="ExternalInput")
    b1_d = nc.dram_tensor("b1p", [128, 4], F32R, kind="ExternalInput")
    b2_d = nc.dram_tensor("b2p", [128, 1], F32R, kind="ExternalInput")
    b3_d = nc.dram_tensor("b3p", [3, 1], F32R, kind="ExternalInput")   # pre-scaled by 0.01
    b3d_d = nc.dram_tensor("b3dp", [128, 4], F32R, kind="ExternalInput")
    ba_d = nc.dram_tensor("bap", [128, 1], F32R, kind="ExternalInput")
    bb_d = nc.dram_tensor("bbp", [64, 1], F32R, kind="ExternalInput")
    bc_d = nc.dram_tensor("bcp", [64, 1], F32R, kind="ExternalInput")
    bd_d = nc.dram_tensor("bdp", [4, 1], F32R, kind="ExternalInput")
    pm_d = nc.dram_tensor("pm", [128, NSLOT], FP32, kind="ExternalInput")
    R_d = nc.dram_tensor("R", [NG, S, T, 512], F32R, kind="ExternalInput")

    x4o_d = nc.dram_tensor("x4o", [3, T], FP32, kind="ExternalOutput")
    prop_d = nc.dram_tensor("propo", [4, 64, T], FP32, kind="ExternalOutput")

    with tile.TileContext(nc) as tc:
        with (
            tc.tile_pool(name="consts", bufs=1) as cpool,
            tc.tile_pool(name="planes", bufs=1) as plpool,
        ):
            # ---- resident constants ----
            w1t_k0 = [cpool.tile([128, 512], F32R, tag=f"w1k0_{dx}", name=f"w1k0_{dx}") for dx in range(3)]
            w1t_k1 = [cpool.tile([72, 512], F32R, tag=f"w1k1_{dx}", name=f"w1k1_{dx}") for dx in range(3)]
            w2t = [cpool.tile([128, 4, 128], F32R, tag=f"w2_{dx}", name=f"w2_{dx}") for dx in range(3)]
            w3t = cpool.tile([128, 3], F32R, tag="w3t", name="w3t")
            wat = cpool.tile([128, 4, 128], F32R, tag="wat", name="wat")
            wbt = cpool.tile([128, 9, 64], F32R, tag="wbt", name="wbt")
            wct = cpool.tile([64, 9, 64], F32R, tag="wct", name="wct")
            wdt = cpool.tile([64, 4], F32R, tag="wdt", name="wdt")
            b1 = cpool.tile([128, 4], F32R, tag="b1", name="b1")
            b2 = cpool.tile([128, 1], F32R, tag="b2", name="b2")
            b3 = cpool.tile([3, 1], F32R, tag="b3", name="b3")
            b3d = cpool.tile([128, 4], F32R, tag="b3d", name="b3d")
            ba = cpool.tile([128, 1], F32R, tag="ba", name="ba")
            bb = cpool.tile([64, 1], F32R, tag="bb", name="bb")
            bc = cpool.tile([64, 1], F32R, tag="bc", name="bc")
            bd = cpool.tile([4, 1], F32R, tag="bd", name="bd")
            pm = cpool.tile([128, NSLOT], FP32, tag="pm", name="pm")

            for dx in range(3):
                nc.sync.dma_start(w1t_k0[dx][:], w1t_d[dx, 0:128, :])
                nc.sync.dma_start(w1t_k1[dx][:], w1t_d[dx, 128:200, :])
                nc.sync.dma_start(
                    w2t[dx][:],
                    w2t_d[dx].rearrange("(k p) c -> p k c", p=128),
                )
            nc.sync.dma_start(w3t[:], w3t_d[:])
            nc.sync.dma_start(wat[:], wat_d.rearrange("(k p) c -> p k c", p=128))
            nc.sync.dma_start(wbt[:], wbt_d.rearrange("k p c -> p k c"))
            nc.sync.dma_start(wct[:], wct_d.rearrange("k p c -> p k c"))
            nc.sync.dma_start(wdt[:], wdt_d[:])
            for t_, d_ in [(b1, b1_d), (b2, b2_d), (b3, b3_d), (b3d, b3d_d),
                           (ba, ba_d), (bb, bb_d), (bc, bc_d), (bd, bd_d),
                           (pm, pm_d)]:
                nc.sync.dma_start(t_[:], d_[:])

            # dense planes (t1-padded columns; zero init)
            p_dense = plpool.tile([128, PLANE + MARGIN], F32R, tag="p_dense", name="p_dense")
            nc.vector.memset(p_dense[:].bitcast(FP32), 0.0)

            # ================= phase 1: front + f + BM + conv_a =================
            with (
                tc.tile_pool(name="front", bufs=1) as fpool,
                tc.tile_pool(name="fsb", bufs=1) as fsb_pool,
                tc.tile_pool(name="rstream", bufs=10) as rpool,
                tc.tile_pool(name="w3dstream", bufs=3) as w3dpool,
                tc.tile_pool(name="cmsb", bufs=6) as cmpool,
                tc.tile_pool(name="psA", bufs=4, space="PSUM") as psA,
                tc.tile_pool(name="psB", bufs=2, space="PSUM") as psB,
            ):
                # ---- front 1D convs ----
                xr0 = fpool.tile([128, COLW], F32R, tag="xr0", name="xr0")
                xr1 = fpool.tile([72, COLW], F32R, tag="xr1", name="xr1")
                nc.gpsimd.memset(xr0[:].bitcast(FP32), 0.0)
                nc.gpsimd.memset(xr1[:].bitcast(FP32), 0.0)
                nc.sync.dma_start(xr0[:, 1:129], x_d[0:128, :])
                nc.sync.dma_start(xr1[:, 1:129], x_d[128:200, :])

                h1 = [fpool.tile([128, COLW], F32R, tag=f"h1_{i}", name=f"h1_{i}") for i in range(4)]
                for i in range(4):
                    nc.gpsimd.memset(h1[i][:].bitcast(FP32), 0.0)
                for ob in range(4):
                    ps = psB.tile([128, 128], FP32, tag="psB", name="psB")
                    first = True
                    for dx in range(3):
                        nc.tensor.matmul(
                            ps[:], w1t_k0[dx][:, ob * 128:(ob + 1) * 128],
                            xr0[:, dx:dx + 128], start=first, stop=False)
                        first = False
                        nc.tensor.matmul(
                            ps[:], w1t_k1[dx][:, ob * 128:(ob + 1) * 128],
                            xr1[:, dx:dx + 128], start=False,
                            stop=(dx == 2))
                    nc.scalar.activation(h1[ob][:, 1:129], ps[:], AF.Relu,
                                         bias=b1[:, ob:ob + 1])

                hps = psB.tile([128, 128], FP32, tag="psB", name="psB")
                for dx in range(3):
                    for kb in range(4):
                        nc.tensor.matmul(
                            hps[:], w2t[dx][:, kb, :], h1[kb][:, dx:dx + 128],
                            start=(dx == 0 and kb == 0),
                            stop=(dx == 2 and kb == 3))
                h = fpool.tile([128, 128], F32R, tag="h", name="h")
                nc.scalar.activation(h[:], hps[:], AF.Relu, bias=b2[:, 0:1])

                # x4 = sigmoid(0.01*(w3 @ h + b3))
                x4ps = psB.tile([3, 128], FP32, tag="psB", name="psB")
                nc.tensor.matmul(x4ps[:], w3t[:], h[:], start=True, stop=True)
                x4sb = fpool.tile([3, 128], FP32, tag="x4sb", name="x4sb")
                nc.scalar.activation(x4sb[:], x4ps[:], AF.Sigmoid,
                                     bias=b3[:, 0:1], scale=0.01)
                nc.sync.dma_start(x4o_d[:], x4sb[:])

                # ---- f_s = h.T-contracted with w3d (layout: t x o) ----
                fsb = [fsb_pool.tile([128, 512], F32R, tag=f"f_{s}", name=f"f_{s}") for s in range(S)]
                for s in range(S):
                    w3ds = w3dpool.tile([128, 512], F32R, tag="w3ds", name="w3ds")
                    nc.sync.dma_start(w3ds[:], w3dt_d[s])
                    fps = psB.tile([128, 512], FP32, tag="psB", name="psB")
                    nc.tensor.matmul(fps[:], h[:], w3ds[:], start=True, stop=True)
                    nc.scalar.activation(fsb[s][:], fps[:], AF.Copy)

                # ---- BM groups + fused conv_a ----
                for g in range(NG):
                    cmps = [psA.tile([128, 512], FP32, tag="cmps", name="cmps")
                            for ob in range(4)]
                    for ph in range(4):
                        rts = []
                        for si in range(8):
                            s = ph * 8 + si
                            rt = rpool.tile([128, 512], F32R, tag="rt", name="rt")
                            nc.sync.dma_start(rt[:], R_d[g, s])
                            rts.append(rt)
                        for ob in range(4):
                            for si in range(8):
                                s = ph * 8 + si
                                nc.tensor.matmul(
                                    cmps[ob][:],
                                    fsb[s][:, ob * 128:(ob + 1) * 128],
                                    rts[si][:],
                                    start=(s == 0), stop=(s == S - 1))
                    cms = []
                    for ob in range(4):
                        cm = cmpool.tile([128, 512], F32R, tag="cm", name="cm")
                        nc.scalar.activation(cm[:], cmps[ob][:], AF.Relu,
                                             bias=b3d[:, ob:ob + 1])
                        cms.append(cm)
                    pps = psB.tile([128, 512], FP32, tag="psB", name="psB")
                    for ob in range(4):
                        nc.tensor.matmul(pps[:], wat[:, ob, :], cms[ob][:],
                                         start=(ob == 0), stop=(ob == 3))
                    # scatter 4 slots into p_dense (cols 1..128 of each slot)
                    nc.scalar.activation(
                        p_dense[:, 4 * g * COLW + 1: 4 * g * COLW + 1 + 4 * COLW]
                        .rearrange("p (sl w) -> p sl w", sl=4)[:, :, 0:128],
                        pps[:].rearrange("p (sl w) -> p sl w", sl=4),
                        AF.Relu, bias=ba[:, 0:1])

                # pad-mask multiply (zeroes out-of-range t2 slots; general-bias safe)
                for sl in range(NSLOT):
                    nc.vector.tensor_scalar_mul(
                        p_dense[:, sl * COLW:(sl + 1) * COLW],
                        p_dense[:, sl * COLW:(sl + 1) * COLW],
                        pm[:, sl:sl + 1])

            # ================= phase 2: 3x3 convs + head =================
            with (
                tc.tile_pool(name="planes2", bufs=1) as pl2,
                tc.tile_pool(name="rwork", bufs=3) as rwork,
                tc.tile_pool(name="psC", bufs=3, space="PSUM") as psC,
            ):
                q_dense = pl2.tile([64, PLANE + MARGIN], F32R, tag="q_dense", name="q_dense")
                nc.vector.memset(q_dense[:].bitcast(FP32), 0.0)
                out_dense = pl2.tile([4, PLANE], FP32, tag="out_dense", name="out_dense")

                # conv_b: q cols [131, 8710)
                qlo, qhi = COLW + 1, (NSLOT - 1) * COLW
                col = qlo
                while col < qhi:
                    n = 512  # tail block padded; garbage lands in margins
                    qps = psC.tile([64, 512], FP32, tag="psC", name="psC")
                    k = 0
                    for ky in range(3):
                        for kx in range(3):
                            off = col + (kx - 1) * COLW + (ky - 1)
                            nc.tensor.matmul(
                                qps[:, 0:n], wbt[:, k, :],
                                p_dense[:, off:off + n],
                                start=(k == 0), stop=(k == 8))
                            k += 1
                    nc.scalar.activation(q_dense[:, col:col + n], qps[:, 0:n],
                                         AF.Relu, bias=bb[:, 0:1])
                    col += n

                # zero q pad columns + pad-mask slots
                for sl in range(NSLOT):
                    nc.vector.tensor_scalar_mul(
                        q_dense[:, sl * COLW:(sl + 1) * COLW],
                        q_dense[:, sl * COLW:(sl + 1) * COLW],
                        pm[:64, sl:sl + 1])
                nc.vector.memset(
                    q_dense[:, 0:PLANE].rearrange("p (sl w) -> p sl w", sl=NSLOT)
                    [:, :, 0:1].bitcast(FP32), 0.0)
                nc.vector.memset(
                    q_dense[:, 0:PLANE].rearrange("p (sl w) -> p sl w", sl=NSLOT)
                    [:, :, 129:130].bitcast(FP32), 0.0)

                # conv_c + conv_d: r cols [261, 8579)
                rlo, rhi = 2 * COLW + 1, (NSLOT - 2) * COLW - 1
                col = rlo
                while col < rhi:
                    n = 512 if rhi - col >= 512 else 128
                    rps = psC.tile([64, 512], FP32, tag="psC", name="psC")
                    k = 0
                    for ky in range(3):
                        for kx in range(3):
                            off = col + (kx - 1) * COLW + (ky - 1)
                            nc.tensor.matmul(
                                rps[:, 0:n], wct[:, k, :],
                                q_dense[:, off:off + n],
                                start=(k == 0), stop=(k == 8))
                            k += 1
                    rsb = rwork.tile([64, 512], F32R, tag="rsb", name="rsb")
                    nc.scalar.activation(rsb[:, 0:n], rps[:, 0:n], AF.Relu,
                                         bias=bc[:, 0:1])
                    dps = psC.tile([4, 512], FP32, tag="psC", name="psC")
                    nc.tensor.matmul(dps[:, 0:n], wdt[:], rsb[:, 0:n],
                                     start=True, stop=True)
                    nc.scalar.activation(out_dense[:, col:col + n], dps[:, 0:n],
                                         AF.Sigmoid, bias=bd[:, 0:1])
                    col += n

                # output DMA: slots 2..65, t1 cols 1..128
                nc.sync.dma_start(
                    prop_d[:],
                    out_dense[:, 2 * COLW:]
                    .rearrange("p (sl w) -> p sl w", sl=NSLOT - 2)[:, 0:64, 1:129])

    nc.finalize()
    return nc


def _prepare_inputs(inputs):
    """Host-side re-layout of weights + mask into per-core in_maps."""
    f32 = np.float32
    x = np.asarray(inputs["x"], f32)
    B = x.shape[0]
    w1 = np.asarray(inputs["w1"], f32)
    w2 = np.asarray(inputs["w2"], f32)
    w3 = np.asarray(inputs["w3"], f32)
    w3d = np.asarray(inputs["w3d"], f32)
    wa = np.asarray(inputs["wa"], f32)
    wb = np.asarray(inputs["wb"], f32)
    wc = np.asarray(inputs["wc"], f32)
    wd = np.asarray(inputs["wd"], f32)
    mask = np.asarray(inputs["sample_mask"], f32)

    def pack_bias(b, nblk, p):
        b = np.asarray(b, f32).reshape(nblk, p).T
        return np.ascontiguousarray(b)

    common = {
        "w1t": np.ascontiguousarray(w1.transpose(2, 1, 0)),
        "w2t": np.ascontiguousarray(w2.transpose(2, 1, 0)),
        "w3t": np.ascontiguousarray(w3[:, :, 0].T),
        "w3dt": np.ascontiguousarray(w3d[:, :, :, 0, 0].transpose(2, 1, 0)),
        "wat": np.ascontiguousarray(wa[:, :, 0, 0].T),
        "wbt": np.ascontiguousarray(wb.transpose(2, 3, 1, 0).reshape(9, 128, 64)),
        "wct": np.ascontiguousarray(wc.transpose(2, 3, 1, 0).reshape(9, 64, 64)),
        "wdt": np.ascontiguousarray(wd[:, :, 0, 0].T),
        "b1p": pack_bias(inputs["b1"], 4, 128),
        "b2p": pack_bias(inputs["b2"], 1, 128),
        "b3p": np.ascontiguousarray(
            0.01 * np.asarray(inputs["b3"], f32).reshape(3, 1)),
        "b3dp": pack_bias(inputs["b3d"], 4, 128),
        "bap": pack_bias(inputs["ba"], 1, 128),
        "bbp": np.asarray(inputs["bb"], f32).reshape(64, 1).copy(),
        "bcp": np.asarray(inputs["bc"], f32).reshape(64, 1).copy(),
        "bdp": np.asarray(inputs["bd"], f32).reshape(4, 1).copy(),
    }

    # mask re-layout: (t, s, t1, t2) -> per-half R[g, s, t, slot4*128 + t1]
    m4 = mask.reshape(T, S, T, T)
    mt = np.ascontiguousarray(m4.transpose(3, 1, 0, 2))  # (t2, s, t, t1)
    halves = []
    for hb in range(2):
        t2s = hb * 64
        Rnp = np.zeros((NG, S, T, 4, T), f32)
        pmv = np.zeros(NSLOT, f32)
        for sl in range(NSLOT):
            t2 = t2s - 2 + sl
            if 0 <= t2 < T:
                Rnp[sl // 4, :, :, sl % 4, :] = mt[t2]
                pmv[sl] = 1.0
        pm = np.broadcast_to(pmv[None, :], (128, NSLOT)).copy()
        halves.append((np.ascontiguousarray(Rnp.reshape(NG, S, T, 512)), pm))

    in_maps = []
    for b in range(B):
        for hb in range(2):
            m = dict(common)
            m["x"] = np.ascontiguousarray(x[b, :200, :])
            m["R"] = halves[hb][0]
            m["pm"] = halves[hb][1]
            in_maps.append(m)
    return in_maps


def kernel(**inputs):
    from concourse.bass_utils import run_bass_kernel_spmd

    if "nc" not in _CACHE:
        _CACHE["nc"] = _build_program()
    nc = _CACHE["nc"]

    in_maps = _prepare_inputs(inputs)
    B = np.asarray(inputs["x"]).shape[0]
    res = run_bass_kernel_spmd(nc, in_maps, core_ids=list(range(len(in_maps))))

    x4 = np.stack([res.results[2 * b]["x4o"] for b in range(B)])  # (B, 3, T)
    prop = np.stack(
        [np.concatenate([res.results[2 * b]["propo"],
                         res.results[2 * b + 1]["propo"]], axis=1)
         for b in range(B)], axis=0)          # (B, 4, t2=128, t1=128)
    prop = np.transpose(prop, (0, 1, 3, 2))   # (B, 4, t1, t2)

    xb_start = x4[:, 0:1, :]
    xb_end = x4[:, 1:2, :]
    xc = x4[:, 2:3, :]
    prop_start = prop[:, 0:1]
    prop_end = prop[:, 1:2]
    iou = prop[:, 2:4]
    return (xc, xb_start, xb_end, iou, prop_start, prop_end)
